# revision 42
# baseline (speedup 1.0000x reference)
"""Longformer encoder (12-layer, sliding-window attention) on 8 Trainium2 cores.

Sharding: (batch=4) x (seq half=2) -> 8 cores; 1024 tokens/core.
Sliding-window attention (+-256) uses a per-layer K/V halo exchange between
the two cores of each batch pair via a 2-rank AllGather.

On-device layout is feature-major: activations [feature_partition, token].
  - projections:  psum[outf, tok] = W[inf, outf].T @ h[inf, tok]   (W stationary)
  - V:            psum[tok, outf] = h[inf, tok].T @ Wv[inf, outf]  (h stationary)
  - scores:       psum[ktok, qtok] = K[hd, ktok].T @ Q[hd, qtok]
  - PV:           psum[hd(+1), qtok] = Vtok[ktok, hd+1].T @ P[ktok, qtok]
                  (extra all-ones column of Vtok yields the softmax denominator)
All matmuls bf16 with fp32 PSUM accumulation; layernorm/softmax math fp32.
"""

import os
import numpy as np
import ml_dtypes

B, S, C = 4, 2048, 128
H, NH, HD, FF, W1 = 768, 12, 64, 3072, 256
L = int(os.environ.get("KERNEL_NL", "12"))
T = 1024            # tokens per core
KP = T + 2 * W1     # padded key range per core (1536)
HT = H // 128       # feature tiles (6)
FT = FF // 128      # ffn feature tiles (24)
VH = HD + 1         # v columns per head incl ones column (65)
VW = NH * VH        # v row width per token tile (780)
EPS = 1e-5
NEG = -30000.0
ISQ = float(1.0 / np.sqrt(HD))

# banded sliding-window geometry: per 512-query block, key tile j (128 keys at
# offset (j-2)*128 from block start) only interacts with queries in
# [QO[j], QO[j]+BWJ[j]) of the block; scores/masks pack at column BOFF[j].
BWJ = [128, 256, 384, 512, 512, 384, 256, 128]
BQO = [0, 0, 0, 0, 0, 128, 256, 384]
BOFF = [0, 128, 384, 768, 1280, 1792, 2176, 2432]
BW = 2560           # banded columns per query block

bf16 = ml_dtypes.bfloat16

_CACHED = {}
LAST_EXEC_NS = None


def _build(n_layers):
    import concourse.bacc as bacc
    import concourse.mybir as mybir
    from concourse import tile
    from contextlib import ExitStack

    dt = mybir.dt
    AF = mybir.ActivationFunctionType
    OP = mybir.AluOpType

    nc = bacc.Bacc(None, target_bir_lowering=False, debug=False)

    # ---------------- DRAM I/O ----------------
    xT = nc.dram_tensor("xT", [C, T], dt.bfloat16, kind="ExternalInput")
    pe = nc.dram_tensor("pe", [H, T], dt.float32, kind="ExternalInput")
    mks = nc.dram_tensor("mks", [128, 2 * BW], dt.bfloat16, kind="ExternalInput")

    up_w1 = nc.dram_tensor("up_w1", [C, H], dt.bfloat16, kind="ExternalInput")
    up_w2 = nc.dram_tensor("up_w2", [H, H], dt.bfloat16, kind="ExternalInput")
    dn_w1 = nc.dram_tensor("dn_w1", [H, H], dt.bfloat16, kind="ExternalInput")
    dn_w2 = nc.dram_tensor("dn_w2", [H, C], dt.bfloat16, kind="ExternalInput")

    Wq = nc.dram_tensor("Wq", [n_layers, H, H], dt.bfloat16, kind="ExternalInput")
    Wk = nc.dram_tensor("Wk", [n_layers, H, H], dt.bfloat16, kind="ExternalInput")
    Wv = nc.dram_tensor("Wv", [n_layers, H, H], dt.bfloat16, kind="ExternalInput")
    Wo = nc.dram_tensor("Wo", [n_layers, H, H], dt.bfloat16, kind="ExternalInput")
    Wi = nc.dram_tensor("Wi", [n_layers, H, FF], dt.bfloat16, kind="ExternalInput")
    Wo2 = nc.dram_tensor("Wo2", [n_layers, FF, H], dt.bfloat16, kind="ExternalInput")

    # per-feature vectors, host-packed as [128, ntiles] (column j = feats 128j:128j+128)
    # order: bq bk bv bo g1 b1 bo2 g2 b2 pad
    vecs = nc.dram_tensor("vecs", [n_layers, 128, 10 * HT], dt.float32, kind="ExternalInput")
    bi_all = nc.dram_tensor("bi_all", [n_layers, 128, FT], dt.float32, kind="ExternalInput")
    # order: up_b1 up_b2 ln_g ln_b dn_b1 pad
    gvec = nc.dram_tensor("gvec", [128, 6 * HT], dt.float32, kind="ExternalInput")
    dn_b2 = nc.dram_tensor("dn_b2", [128, 1], dt.float32, kind="ExternalInput")

    y = nc.dram_tensor("y", [C, T], dt.float32, kind="ExternalOutput")

    # halo exchange buffers: h-stream edges (halo K/V projected locally)
    KSEG = HT * 128 * W1
    SEND_N = 2 * KSEG
    cc_send = nc.dram_tensor("cc_send", [SEND_N], dt.bfloat16)
    cc_recv = nc.dram_tensor("cc_recv", [2, SEND_N], dt.bfloat16)
    K_OFF = [0, KSEG]

    with tile.TileContext(nc) as tc, ExitStack() as ctx:
        pp = ctx.enter_context(tc.tile_pool(name="persist", bufs=1))
        wp = ctx.enter_context(tc.tile_pool(name="w768", bufs=6))
        bp = ctx.enter_context(tc.tile_pool(name="bias", bufs=2))
        fp = ctx.enter_context(tc.tile_pool(name="ffmid", bufs=2))
        sp = ctx.enter_context(tc.tile_pool(name="scratch", bufs=1))
        rp = ctx.enter_context(tc.tile_pool(name="rows", bufs=2))
        pbp = ctx.enter_context(tc.tile_pool(name="pbuf", bufs=3))
        psA = ctx.enter_context(tc.tile_pool(name="psA", bufs=2, space="PSUM"))

        # ---------------- persistent tiles ----------------
        h = pp.tile([128, HT * T], dt.float32, tag="h")          # residual stream
        hb = pp.tile([128, HT * T], dt.bfloat16, tag="hb")       # bf16 copy of stream
        qb = pp.tile([128, HT * T], dt.bfloat16, tag="qb")       # Q (feature-major)
        ob = pp.tile([128, HT * T], dt.bfloat16, tag="ob")       # attn out (feature-major)
        # K padded, feature-major, split [left pad | interior | right pad]
        Kpl = pp.tile([128, HT * W1], dt.bfloat16, tag="Kpl")
        Kpm = pp.tile([128, HT * T], dt.bfloat16, tag="Kpm")
        Kpr = pp.tile([128, HT * W1], dt.bfloat16, tag="Kpr")
        # V padded, token-major (65-wide head slots), split by token tiles 0-1|2-9|10-11
        Vpl = pp.tile([128, 2 * VW], dt.bfloat16, tag="Vpl")
        Vpm = pp.tile([128, 8 * VW], dt.bfloat16, tag="Vpm")
        Vpr = pp.tile([128, 2 * VW], dt.bfloat16, tag="Vpr")
        hbh = pp.tile([128, HT * 512], dt.bfloat16, tag="hbh")   # halo h [left|right]
        msk = pp.tile([128, 2 * BW], dt.bfloat16, tag="msk")     # banded additive masks
        ones_c = pp.tile([128, 1], dt.bfloat16, tag="ones_c")    # [128,1] ones (stats lhsT)
        ones_r = pp.tile([128, 128], dt.bfloat16, tag="ones_r")  # ones (bcast lhsT slices)
        gv = pp.tile([128, 6 * HT], dt.float32, tag="gv")
        dnb2 = pp.tile([128, 1], dt.float32, tag="dnb2")
        epsc = pp.tile([128, 1], dt.float32, tag="epsc")

        def hs(ft, qt=None):
            if qt is None:
                return slice(ft * T, (ft + 1) * T)
            return slice(ft * T + qt * 512, ft * T + qt * 512 + 512)

        def k_ap(ft, kt):  # lhsT AP [128, 128] for padded key tile kt (0..11)
            if kt < 2:
                return Kpl[:, ft * W1 + kt * 128: ft * W1 + (kt + 1) * 128]
            if kt < 10:
                return Kpm[:, ft * T + (kt - 2) * 128: ft * T + (kt - 1) * 128]
            return Kpr[:, ft * W1 + (kt - 10) * 128: ft * W1 + (kt - 9) * 128]

        def v_ap(i):  # full [128, VW] token tile i of padded V (0..11)
            if i < 2:
                return Vpl[:, i * VW:(i + 1) * VW]
            if i < 10:
                return Vpm[:, (i - 2) * VW:(i - 1) * VW]
            return Vpr[:, (i - 10) * VW:(i - 9) * VW]

        nc.vector.memset(ones_c[:], 1.0)
        nc.vector.memset(ones_r[:], 1.0)
        nc.vector.memset(epsc[:], EPS)
        for i in range(12):
            vre = v_ap(i).rearrange("p (n c) -> p n c", c=VH)
            nc.vector.memset(vre[:, :, HD:HD + 1], 1.0)

        nc.sync.dma_start(out=gv[:], in_=gvec[:])
        nc.sync.dma_start(out=dnb2[:], in_=dn_b2[:])
        nc.sync.dma_start(out=msk[:], in_=mks[:])

        # ---------------- helpers ----------------
        def w_proj(w_dram, rhs_cols, out_fn, n_k=HT, nq=2, qt_post=None,
                   qt_mid=None):
            """psum[mt][qt] = W.T @ rhs ; out_fn(mt, qt, psum) evicts.
            qt-outer so per-half hooks can overlap the other half."""
            wt = []
            for k in range(n_k):
                t = wp.tile([128, H], dt.bfloat16, tag="w768", bufs=8)
                nc.sync.dma_start(out=t[:], in_=w_dram[k * 128:(k + 1) * 128, :])
                wt.append(t)
            for qt in range(nq):
                for mt in range(HT):
                    pt = psA.tile([128, 512], dt.float32, tag="work")
                    for k in range(n_k):
                        nc.tensor.matmul(
                            pt[:], wt[k][:, mt * 128:(mt + 1) * 128],
                            rhs_cols(k, qt),
                            start=(k == 0), stop=(k == n_k - 1))
                    out_fn(mt, qt, pt)
                    if qt_mid is not None:
                        qt_mid(qt, mt)
                if qt_post is not None:
                    qt_post(qt)

        # layernorm for one 512-token half, split into stats (serial scalar
        # chain) and apply (PE broadcasts + row updates) so the chain can hide
        # behind unrelated PE work emitted between the two parts.
        # rsqrt via Act Sqrt + DVE fast reciprocal (single act-table function).
        # h gets g*rsqrt*(h-mean); +b lands on hb via act bias and on h via
        # the Pool engine off the critical path.
        def ln_stats(qt, tS):
            ubf = sp.tile([128, HT * T], dt.bfloat16, tag="scr")
            ub = ubf[:, qt * HT * 512:(qt + 1) * HT * 512]
            for ft in range(HT):
                nc.scalar.activation(ub[:, ft * 512:(ft + 1) * 512],
                                     h[:, hs(ft, qt)], AF.Copy)
            statm = tS[0:1, :]
            for ft in range(HT):
                nc.tensor.matmul(statm, ones_c[:], ub[:, ft * 512:(ft + 1) * 512],
                                 start=(ft == 0), stop=(ft == HT - 1))
            mrow = rp.tile([1, 512], dt.bfloat16, tag="mrow")
            nc.scalar.activation(mrow[:], statm, AF.Copy, scale=1.0 / H)
            m2 = rp.tile([1, 512], dt.float32, tag="rowA")
            nc.scalar.activation(m2[:], statm, AF.Square, scale=1.0 / H)
            for ft in range(HT):
                nc.scalar.square(ub[:, ft * 512:(ft + 1) * 512],
                                 ub[:, ft * 512:(ft + 1) * 512])
            sqs = tS[32:33, :]
            for ft in range(HT):
                nc.tensor.matmul(sqs, ones_c[:], ub[:, ft * 512:(ft + 1) * 512],
                                 start=(ft == 0), stop=(ft == HT - 1))
            var = rp.tile([1, 512], dt.float32, tag="rowB")
            nc.vector.scalar_tensor_tensor(var[:], sqs, 1.0 / H, m2[:],
                                           OP.mult, OP.subtract)
            sd = rp.tile([1, 512], dt.float32, tag="rowA", name="sd")
            nc.scalar.activation(sd[:], var[:], AF.Sqrt, bias=epsc[0:1, :])
            rf = rp.tile([1, 512], dt.float32, tag="rowB", name="rf")
            nc.vector.reciprocal_approx_fast(rf[:], sd[:])
            rrow = rp.tile([1, 512], dt.bfloat16, tag="mrow", name="rrow")
            nc.vector.tensor_copy(rrow[:], rf[:])
            m2row = rp.tile([1, 512], dt.bfloat16, tag="rowC", name="m2row")
            nc.vector.tensor_mul(m2row[:], mrow[:], rrow[:])
            return rrow, m2row

        def ln_apply(qt, tA, tB, rows, g_col, ng_col, b_col):
            rrow, m2row = rows
            nc.tensor.matmul(tB[:], ones_r[0:1, :], rrow[:], start=True, stop=True)
            nc.tensor.matmul(tA[:], ones_r[0:1, :], m2row[:], start=True, stop=True)
            # evict broadcasts to SBUF right away so the psum slots free fast
            rbs = rp.tile([128, 512], dt.bfloat16, tag="lnrb")
            nc.scalar.activation(rbs[:], tB[:], AF.Copy)
            mgs = rp.tile([128, 512], dt.bfloat16, tag="lnmg")
            nc.scalar.activation(mgs[:], tA[:], AF.Copy)
            for ft in range(HT):
                sl = hs(ft, qt)
                tmp = rp.tile([128, 512], dt.float32, tag="lntmp")
                nc.vector.scalar_tensor_tensor(tmp[:], h[:, sl], g_col(ft),
                                               rbs[:], OP.mult, OP.mult)
                nc.vector.scalar_tensor_tensor(h[:, sl], mgs[:], ng_col(ft),
                                               tmp[:], OP.mult, OP.add)
                nc.scalar.activation(hb[:, sl], h[:, sl], AF.Identity,
                                     bias=b_col(ft))
            for ft in range(HT):
                sl = hs(ft, qt)
                nc.vector.tensor_scalar_add(h[:, sl], h[:, sl], b_col(ft))

        # ---------------- input projection ----------------
        # h <- pos_emb + t_emb  (host-combined), then += up-proj
        for ft in range(HT):
            nc.sync.dma_start(out=h[:, hs(ft)], in_=pe[ft * 128:(ft + 1) * 128, :])
        xb = sp.tile([128, T], dt.bfloat16, tag="xb")
        nc.sync.dma_start(out=xb[:], in_=xT[:])

        w1t = wp.tile([128, H], dt.bfloat16, tag="w768", bufs=8)
        nc.sync.dma_start(out=w1t[:], in_=up_w1[:])
        t1 = sp.tile([128, HT * T], dt.bfloat16, tag="scr")
        for mt in range(HT):
            for qt in range(2):
                pt = psA.tile([128, 512], dt.float32, tag="work")
                nc.tensor.matmul(pt[:], w1t[:, mt * 128:(mt + 1) * 128],
                                 xb[:, qt * 512:qt * 512 + 512], start=True, stop=True)
                nc.scalar.activation(t1[:, hs(mt, qt)], pt[:], AF.Tanh,
                                     bias=gv[:, 0 * HT + mt:0 * HT + mt + 1])

        def up2_out(mt, qt, pt):
            sl = hs(mt, qt)
            nc.vector.scalar_tensor_tensor(
                h[:, sl], pt[:], gv[:, 1 * HT + mt:1 * HT + mt + 1], h[:, sl],
                OP.add, OP.add)
        w_proj(up_w2, lambda k, qt: t1[:, hs(k, qt)], up2_out)

        with tc.tile_pool(name="lnps", bufs=2, space="PSUM") as lp0:
            g0 = lambda ft: gv[:, 2 * HT + ft:2 * HT + ft + 1]
            b0 = lambda ft: gv[:, 3 * HT + ft:3 * HT + ft + 1]
            ng0 = lambda ft: gv[:, 5 * HT + ft:5 * HT + ft + 1]
            for qt in range(2):
                tS = lp0.tile([128, 512], dt.float32, tag="lnS")
                rows = ln_stats(qt, tS)
                tA = lp0.tile([128, 512], dt.float32, tag="lnA")
                tB = lp0.tile([128, 512], dt.float32, tag="lnB")
                ln_apply(qt, tA, tB, rows, g0, ng0, b0)

        # deferred LN apply carried across phase boundaries (hides the serial
        # stats->rsqrt scalar chain behind unrelated PE work)
        pend = [None]

        def flush_pend():
            if pend[0] is not None:
                f, pend[0] = pend[0], None
                f()

        # ---------------- encoder layers ----------------
        for l in range(n_layers):
            bv_t = bp.tile([128, 10 * HT], dt.float32, tag="bvec")
            nc.sync.dma_start(out=bv_t[:], in_=vecs[l])
            bi_t = bp.tile([128, FT], dt.float32, tag="bivec")
            nc.sync.dma_start(out=bi_t[:], in_=bi_all[l])

            def vcol(i, ft):
                return bv_t[:, i * HT + ft:i * HT + ft + 1]

            # --- h-edge halo exchange (overlaps with Q/K/V projections) ---
            def emit_halo():
                for ft in range(HT):
                    nc.sync.dma_start(
                        out=cc_send[K_OFF[0] + ft * 128 * W1:K_OFF[0] + (ft + 1) * 128 * W1]
                        .rearrange("(p t) -> p t", p=128),
                        in_=hb[:, ft * T:ft * T + W1])
                    nc.sync.dma_start(
                        out=cc_send[K_OFF[1] + ft * 128 * W1:K_OFF[1] + (ft + 1) * 128 * W1]
                        .rearrange("(p t) -> p t", p=128),
                        in_=hb[:, ft * T + T - W1:(ft + 1) * T])
                nc.gpsimd.collective_compute(
                    "AllGather", OP.bypass, ins=[cc_send[:]], outs=[cc_recv[:]],
                    replica_groups=[[0, 1], [2, 3], [4, 5], [6, 7]])
                # left halo <- rank0's right edge ; right halo <- rank1's left edge
                for ft in range(HT):
                    nc.sync.dma_start(
                        out=hbh[:, ft * 512:ft * 512 + W1],
                        in_=cc_recv[0, K_OFF[1] + ft * 128 * W1:K_OFF[1] + (ft + 1) * 128 * W1]
                        .rearrange("(p t) -> p t", p=128))
                    nc.sync.dma_start(
                        out=hbh[:, ft * 512 + W1:(ft + 1) * 512],
                        in_=cc_recv[1, K_OFF[0] + ft * 128 * W1:K_OFF[0] + (ft + 1) * 128 * W1]
                        .rearrange("(p t) -> p t", p=128))

            # --- Q, K projections (feature-major) ---
            def q_out(mt, qt, pt):
                nc.vector.tensor_scalar_add(qb[:, hs(mt, qt)], pt[:], vcol(0, mt))

            def q_post(qt):
                if qt == 0:
                    flush_pend()
                    emit_halo()
            w_proj(Wq[l], lambda k, qt: hb[:, hs(k, qt)], q_out, qt_post=q_post)

            def k_out(mt, qt, pt):
                if qt < 2:
                    sl = slice(mt * T + qt * 512, mt * T + qt * 512 + 512)
                    nc.vector.tensor_scalar_add(Kpm[:, sl], pt[:], vcol(1, mt))
                else:
                    nc.vector.tensor_scalar_add(Kpl[:, mt * W1:(mt + 1) * W1],
                                                pt[:, 0:W1], vcol(1, mt))
                    nc.vector.tensor_scalar_add(Kpr[:, mt * W1:(mt + 1) * W1],
                                                pt[:, W1:512], vcol(1, mt))

            def k_rhs(k, qt):
                if qt < 2:
                    return hb[:, hs(k, qt)]
                return hbh[:, k * 512:(k + 1) * 512]
            w_proj(Wk[l], k_rhs, k_out, nq=3)

            # --- V projection (token-major: h stationary) ---
            wvt = []
            for k in range(HT):
                t = wp.tile([128, H], dt.bfloat16, tag="w768", bufs=8)
                nc.sync.dma_start(out=t[:], in_=Wv[l, k * 128:(k + 1) * 128, :])
                wvt.append(t)
            def v_stat(vt, k):  # stationary h slice for padded v token tile vt
                if vt < 2:
                    return hbh[:, k * 512 + vt * 128:k * 512 + (vt + 1) * 128]
                if vt < 10:
                    tt = vt - 2
                    return hb[:, k * T + tt * 128:k * T + tt * 128 + 128]
                return hbh[:, k * 512 + W1 + (vt - 10) * 128:k * 512 + W1 + (vt - 9) * 128]

            for vt in (2, 3, 4, 5, 6, 7, 8, 9, 0, 1, 10, 11):  # interior first
                for n0, nn in ((0, 512), (512, 256)):
                    pt = psA.tile([128, 512], dt.float32, tag="work")
                    for k in range(HT):
                        nc.tensor.matmul(
                            pt[:, :nn], v_stat(vt, k), wvt[k][:, n0:n0 + nn],
                            start=(k == 0), stop=(k == HT - 1))
                    dst = v_ap(vt).rearrange("p (n c) -> p n c", c=VH)
                    h0, nh_ = n0 // HD, nn // HD
                    src = pt[:, :nn].rearrange("p (n c) -> p n c", c=HD)
                    nc.scalar.activation(dst[:, h0:h0 + nh_, 0:HD], src[:], AF.Copy)

            # --- attention (banded, deferred softmax normalization) ---
            # ob accumulates UNNORMALIZED per-head PV; per-query reciprocal
            # denominators are broadcast per head-pair and multiplied in after.
            # bv's contribution is folded into bo on the host (bo_eff).
            att_cm = tc.tile_pool(name="attps", bufs=2, space="PSUM")
            att_ps = att_cm.__enter__()
            dnp_cm = tc.tile_pool(name="dnp", bufs=2)
            dnp = dnp_cm.__enter__()
            for qt in range(2):
                for ft in range(HT):
                    rbp = psA.tile([128, 512], dt.float32, tag="work")
                    for r in range(2):
                        hd_i, r0 = 2 * ft + r, r * HD
                        pb = pbp.tile([128, BW], dt.bfloat16, tag="pb")
                        for j in range(8):
                            w, off = BWJ[j], BOFF[j]
                            q0 = ft * T + qt * 512 + BQO[j]
                            sc = att_ps.tile([128, 512], dt.float32, tag="sc",
                                             bufs=4)
                            nc.tensor.matmul(
                                sc[:, 0:w], k_ap(ft, 4 * qt + j)[r0:r0 + HD, :],
                                qb[r0:r0 + HD, q0:q0 + w],
                                start=True, stop=True)
                            nc.vector.tensor_add(
                                pb[:, off:off + w], sc[:, 0:w],
                                msk[:, qt * BW + off:qt * BW + off + w])
                        nc.scalar.activation(pb[:], pb[:], AF.Exp, scale=ISQ)
                        pv = att_ps.tile([VH, 512], dt.float32, tag="pvb", bufs=2)
                        for j in (3, 4, 0, 1, 2, 5, 6, 7):
                            nc.tensor.matmul(
                                pv[:, BQO[j]:BQO[j] + BWJ[j]],
                                v_ap(4 * qt + j)[:, hd_i * VH:(hd_i + 1) * VH],
                                pb[:, BOFF[j]:BOFF[j] + BWJ[j]],
                                start=(j == 3), stop=(j == 7),
                                skip_group_check=True)
                        dn_i = dnp.tile([1, 512], dt.float32, tag="dn_i")
                        nc.scalar.activation(dn_i[:], pv[HD:VH, :], AF.Copy)
                        nc.scalar.activation(ob[r0:r0 + HD, hs(ft, qt)],
                                             pv[0:HD, :], AF.Copy)
                        dn_f = dnp.tile([1, 512], dt.float32, tag="dn_f")
                        nc.vector.reciprocal_approx_fast(dn_f[:], dn_i[:])
                        dn_r = dnp.tile([1, 512], dt.bfloat16, tag="dn_r")
                        nc.vector.tensor_copy(dn_r[:], dn_f[:])
                        nc.tensor.matmul(rbp[r0:r0 + HD, :], ones_r[0:1, 0:HD],
                                         dn_r[:], start=True, stop=True)
                    nc.vector.tensor_mul(ob[:, hs(ft, qt)], ob[:, hs(ft, qt)],
                                         rbp[:])
            dnp_cm.__exit__(None, None, None)
            att_cm.__exit__(None, None, None)

            # --- O projection + residual + LN1, FFN + LN2 ---
            # LN stat rows live in spare partitions of ff2_* psum slots; LN
            # applies run from psA between phases, so the serial chains hide
            # behind the other half's projections / FFN chunks.
            ffn_cm = tc.tile_pool(name="ffps", bufs=1, space="PSUM")
            ffn_ps = ffn_cm.__enter__()

            g1c = lambda ft: vcol(4, ft)
            ng1c = lambda ft: vcol(2, ft)
            b1c = lambda ft: vcol(5, ft)
            g2c = lambda ft: vcol(7, ft)
            ng2c = lambda ft: vcol(9, ft)
            b2c = lambda ft: vcol(8, ft)

            def ln_apply_psA(qt, rows, gc, ngc, bc):
                tA = psA.tile([128, 512], dt.float32, tag="work", name="lnA")
                tB = psA.tile([128, 512], dt.float32, tag="work", name="lnB")
                ln_apply(qt, tA, tB, rows, gc, ngc, bc)

            ln1_rows = {}

            def o_out(mt, qt, pt):
                sl = hs(mt, qt)
                nc.vector.scalar_tensor_tensor(h[:, sl], pt[:], vcol(3, mt), h[:, sl],
                                               OP.add, OP.add)

            def o_post(qt):
                tS = ffn_ps.tile([128, 512], dt.float32,
                                 tag=f"ff2_{2 + qt}", name=f"ln1S_{qt}")
                ln1_rows[qt] = ln_stats(qt, tS)
                if qt == 1:
                    pend[0] = lambda: ln_apply_psA(1, ln1_rows[1], g1c, ng1c, b1c)

            def o_mid(qt, mt):
                if qt == 1 and mt == 1:
                    ln_apply_psA(0, ln1_rows[0], g1c, ng1c, b1c)
            w_proj(Wo[l], lambda k, qt: ob[:, hs(k, qt)], o_out,
                   qt_post=o_post, qt_mid=o_mid)

            # --- FFN ---
            ln2_rows = {}
            for qt in range(2):
                fpt = [ffn_ps.tile([128, 512], dt.float32, tag=f"ff2_{m}", name=f"ff2_{m}") for m in range(HT)]
                for ch in range(6):  # 6 chunks of 4 mid tiles (512 cols of FF)
                    wi_ch = []
                    for k in range(HT):
                        t = wp.tile([128, 512], dt.bfloat16, tag="wi", bufs=8)
                        nc.sync.dma_start(
                            out=t[:],
                            in_=Wi[l, k * 128:(k + 1) * 128, ch * 512:(ch + 1) * 512])
                        wi_ch.append(t)
                    fmid = fp.tile([128, 4 * 512], dt.bfloat16, tag="fmid")
                    for mi in range(4):
                        mt = ch * 4 + mi
                        pt = psA.tile([128, 512], dt.float32, tag="work")
                        for k in range(HT):
                            nc.tensor.matmul(
                                pt[:], wi_ch[k][:, mi * 128:(mi + 1) * 128],
                                hb[:, hs(k, qt)],
                                start=(k == 0), stop=(k == HT - 1))
                        nc.scalar.activation(fmid[:, mi * 512:(mi + 1) * 512], pt[:],
                                             AF.Gelu, bias=bi_t[:, mt:mt + 1])
                    wo2_ch = []
                    for mi in range(4):
                        t = wp.tile([128, H], dt.bfloat16, tag="w768", bufs=8)
                        nc.sync.dma_start(
                            out=t[:],
                            in_=Wo2[l, (ch * 4 + mi) * 128:(ch * 4 + mi + 1) * 128, :])
                        wo2_ch.append(t)
                    for m in range(HT):
                        for mi in range(4):
                            kt = ch * 4 + mi
                            nc.tensor.matmul(
                                fpt[m][:], wo2_ch[mi][:, m * 128:(m + 1) * 128],
                                fmid[:, mi * 512:(mi + 1) * 512],
                                start=(kt == 0), stop=(kt == FT - 1))
                    if ch == 1:
                        if qt == 0:
                            flush_pend()  # LN1 qt1 apply
                        else:
                            ln_apply_psA(0, ln2_rows[0], g2c, ng2c, b2c)
                for m in range(HT):
                    sl = hs(m, qt)
                    nc.vector.scalar_tensor_tensor(h[:, sl], fpt[m][:], vcol(6, m),
                                                   h[:, sl], OP.add, OP.add)
                tS = ffn_ps.tile([128, 512], dt.float32,
                                 tag=f"ff2_{2 + qt}", name=f"ln2S_{qt}")
                ln2_rows[qt] = ln_stats(qt, tS)
            lq1, lb = ln2_rows[1], bv_t
            pend[0] = (lambda rows=lq1, bt=lb:
                       ln_apply_psA(1, rows,
                                    lambda ft: bt[:, 7 * HT + ft:7 * HT + ft + 1],
                                    lambda ft: bt[:, 9 * HT + ft:9 * HT + ft + 1],
                                    lambda ft: bt[:, 8 * HT + ft:8 * HT + ft + 1]))
            ffn_cm.__exit__(None, None, None)

        # ---------------- output projection ----------------
        flush_pend()
        t2 = sp.tile([128, HT * T], dt.bfloat16, tag="scr")

        def d1_out(mt, qt, pt):
            nc.scalar.activation(t2[:, hs(mt, qt)], pt[:], AF.Tanh,
                                 bias=gv[:, 4 * HT + mt:4 * HT + mt + 1])
        w_proj(dn_w1, lambda k, qt: hb[:, hs(k, qt)], d1_out)

        w2t = wp.tile([128, HT * C], dt.bfloat16, tag="w768", bufs=8)
        for k in range(HT):
            nc.sync.dma_start(out=w2t[:, k * C:(k + 1) * C],
                              in_=dn_w2[k * 128:(k + 1) * 128, :])
        for qt in range(2):
            pt = psA.tile([128, 512], dt.float32, tag="work")
            for k in range(HT):
                nc.tensor.matmul(pt[:], w2t[:, k * C:(k + 1) * C],
                                 t2[:, hs(k, qt)], start=(k == 0), stop=(k == HT - 1))
            yo = rp.tile([128, 512], dt.float32, tag="lntmp", name="yout")
            nc.scalar.activation(yo[:], pt[:], AF.Identity, bias=dnb2[:])
            nc.sync.dma_start(out=y[:, qt * 512:qt * 512 + 512], in_=yo[:])

    nc.compile()
    return nc


def _host_prep(inputs, n_layers):
    f32 = np.float32
    x = np.asarray(inputs["x"], f32)
    ts = np.asarray(inputs["timesteps"])
    half = C // 2
    freqs = np.exp(-np.log(10000.0) * np.arange(half, dtype=f32) / half)
    a = ts.astype(f32)[:, None] * freqs[None, :]
    emb0 = np.concatenate([np.cos(a), np.sin(a)], -1).astype(f32)
    t1 = emb0 @ np.asarray(inputs["t_w1"], f32) + np.asarray(inputs["t_b1"], f32)
    t1 = t1 / (1.0 + np.exp(-t1))
    emb = (t1 @ np.asarray(inputs["t_w2"], f32) + np.asarray(inputs["t_b2"], f32)).astype(f32)

    def cvt(w):
        return np.ascontiguousarray(np.asarray(w, f32).astype(bf16))

    def packvec(v, nt):
        return np.ascontiguousarray(np.asarray(v, f32).reshape(nt, 128).T)

    com = dict(
        up_w1=cvt(inputs["up_w1"]), up_w2=cvt(inputs["up_w2"]),
        dn_w1=cvt(inputs["down_w1"]), dn_w2=cvt(inputs["down_w2"]),
        Wq=cvt(inputs["Wq"][:n_layers]), Wk=cvt(inputs["Wk"][:n_layers]),
        Wv=cvt(inputs["Wv"][:n_layers]), Wo=cvt(inputs["Wo"][:n_layers]),
        Wi=cvt(inputs["Wi"][:n_layers]), Wo2=cvt(inputs["Wo2"][:n_layers]),
        dn_b2=np.ascontiguousarray(np.asarray(inputs["down_b2"], f32).reshape(1, C).T),
    )
    # bo slot carries bo + bv @ Wo: the attention out bias bv commutes with
    # softmax normalization and folds through the O projection.
    bo_eff = (np.asarray(inputs["bo"], f32)[:n_layers]
              + np.einsum("lh,lhk->lk", np.asarray(inputs["bv"], f32)[:n_layers],
                          np.asarray(inputs["Wo"], f32)[:n_layers]))
    lvec = {k: np.asarray(inputs[k], f32) for k in
            ("bq", "bk", "g1", "b1", "bo2", "g2", "b2")}
    lvec["bo"] = bo_eff
    lvec["ng1"] = -lvec["g1"]
    lvec["ng2"] = -lvec["g2"]
    vecs = np.stack([
        np.concatenate([packvec(lvec[k][l], HT) for k in
                        ("bq", "bk", "ng1", "bo", "g1", "b1", "bo2", "g2", "b2", "ng2")],
                       axis=1)
        for l in range(n_layers)])
    com["vecs"] = np.ascontiguousarray(vecs.astype(f32))
    com["bi_all"] = np.ascontiguousarray(
        np.stack([packvec(np.asarray(inputs["bi"], f32)[l], FT)
                  for l in range(n_layers)]).astype(f32))
    com["gvec"] = np.ascontiguousarray(np.concatenate([
        packvec(inputs["up_b1"], HT), packvec(inputs["up_b2"], HT),
        packvec(inputs["ln_g"], HT), packvec(inputs["ln_b"], HT),
        packvec(inputs["down_b1"], HT),
        packvec(-np.asarray(inputs["ln_g"], f32), HT)],
        axis=1).astype(f32))

    pos = np.asarray(inputs["pos_emb"], f32)
    mk = {}
    for sh in range(2):
        base = sh * T
        m = np.empty((128, 2 * BW), f32)
        for qt in range(2):
            for j in range(8):
                gk = base - W1 + (4 * qt + j) * 128 + np.arange(128)[:, None]
                gq = base + qt * 512 + BQO[j] + np.arange(BWJ[j])[None, :]
                valid = (np.abs(gk - gq) <= W1) & (gk >= 0) & (gk < S)
                m[:, qt * BW + BOFF[j]:qt * BW + BOFF[j] + BWJ[j]] = \
                    np.where(valid, 0.0, NEG)
        mk[sh] = np.ascontiguousarray(m.astype(bf16))

    in_maps = []
    for c in range(8):
        b, sh = c // 2, c % 2
        sl = slice(sh * T, (sh + 1) * T)
        im = dict(com)
        im["xT"] = np.ascontiguousarray(x[b, sl].T.astype(bf16))
        im["pe"] = np.ascontiguousarray((pos[sl] + emb[b][None, :]).T.astype(f32))
        im["mks"] = mk[sh]
        in_maps.append(im)
    return in_maps


def kernel(**inputs):
    from concourse.bass_utils import run_bass_kernel_spmd

    n_layers = L
    if n_layers not in _CACHED:
        _CACHED[n_layers] = _build(n_layers)
    nc = _CACHED[n_layers]
    in_maps = _host_prep(inputs, n_layers)
    trace = os.environ.get("KERNEL_TRACE", "0") == "1"
    tmpdir = os.environ.get("KERNEL_TMPDIR") or None
    res = run_bass_kernel_spmd(nc, in_maps, list(range(8)), trace=trace,
                               tmpdir=tmpdir)
    global LAST_EXEC_NS
    if getattr(res, "exec_time_ns", None):
        LAST_EXEC_NS = res.exec_time_ns
    out = np.empty((B, S, C), np.float32)
    for c in range(8):
        b, sh = c // 2, c % 2
        out[b, sh * T:(sh + 1) * T, :] = res.results[c]["y"].T
    return out



# revision 45
# speedup vs baseline: 1.0045x; 1.0045x over previous
"""Longformer encoder (12-layer, sliding-window attention) on 8 Trainium2 cores.

Sharding: (batch=4) x (seq half=2) -> 8 cores; 1024 tokens/core.
Sliding-window attention (+-256) uses a per-layer K/V halo exchange between
the two cores of each batch pair via a 2-rank AllGather.

On-device layout is feature-major: activations [feature_partition, token].
  - projections:  psum[outf, tok] = W[inf, outf].T @ h[inf, tok]   (W stationary)
  - V:            psum[tok, outf] = h[inf, tok].T @ Wv[inf, outf]  (h stationary)
  - scores:       psum[ktok, qtok] = K[hd, ktok].T @ Q[hd, qtok]
  - PV:           psum[hd(+1), qtok] = Vtok[ktok, hd+1].T @ P[ktok, qtok]
                  (extra all-ones column of Vtok yields the softmax denominator)
All matmuls bf16 with fp32 PSUM accumulation; layernorm/softmax math fp32.
"""

import os
import numpy as np
import ml_dtypes

B, S, C = 4, 2048, 128
H, NH, HD, FF, W1 = 768, 12, 64, 3072, 256
L = int(os.environ.get("KERNEL_NL", "12"))
T = 1024            # tokens per core
KP = T + 2 * W1     # padded key range per core (1536)
HT = H // 128       # feature tiles (6)
FT = FF // 128      # ffn feature tiles (24)
VH = HD + 1         # v columns per head incl ones column (65)
VW = NH * VH        # v row width per token tile (780)
EPS = 1e-5
NEG = -30000.0
ISQ = float(1.0 / np.sqrt(HD))

# banded sliding-window geometry: per 512-query block, key tile j (128 keys at
# offset (j-2)*128 from block start) only interacts with queries in
# [QO[j], QO[j]+BWJ[j]) of the block; scores/masks pack at column BOFF[j].
BWJ = [128, 256, 384, 512, 512, 384, 256, 128]
BQO = [0, 0, 0, 0, 0, 128, 256, 384]
BOFF = [0, 128, 384, 768, 1280, 1792, 2176, 2432]
BW = 2560           # banded columns per query block

bf16 = ml_dtypes.bfloat16

_CACHED = {}
LAST_EXEC_NS = None


def _build(n_layers):
    import concourse.bacc as bacc
    import concourse.mybir as mybir
    from concourse import tile
    from contextlib import ExitStack

    dt = mybir.dt
    AF = mybir.ActivationFunctionType
    OP = mybir.AluOpType

    nc = bacc.Bacc(None, target_bir_lowering=False, debug=False)

    # ---------------- DRAM I/O ----------------
    xT = nc.dram_tensor("xT", [C, T], dt.bfloat16, kind="ExternalInput")
    pe = nc.dram_tensor("pe", [H, T], dt.float32, kind="ExternalInput")
    mks = nc.dram_tensor("mks", [128, 2 * BW], dt.bfloat16, kind="ExternalInput")

    up_w1 = nc.dram_tensor("up_w1", [C, H], dt.bfloat16, kind="ExternalInput")
    up_w2 = nc.dram_tensor("up_w2", [H, H], dt.bfloat16, kind="ExternalInput")
    dn_w1 = nc.dram_tensor("dn_w1", [H, H], dt.bfloat16, kind="ExternalInput")
    dn_w2 = nc.dram_tensor("dn_w2", [H, C], dt.bfloat16, kind="ExternalInput")

    Wq = nc.dram_tensor("Wq", [n_layers, H, H], dt.bfloat16, kind="ExternalInput")
    Wk = nc.dram_tensor("Wk", [n_layers, H, H], dt.bfloat16, kind="ExternalInput")
    Wv = nc.dram_tensor("Wv", [n_layers, H, H], dt.bfloat16, kind="ExternalInput")
    Wo = nc.dram_tensor("Wo", [n_layers, H, H], dt.bfloat16, kind="ExternalInput")
    Wi = nc.dram_tensor("Wi", [n_layers, H, FF], dt.bfloat16, kind="ExternalInput")
    Wo2 = nc.dram_tensor("Wo2", [n_layers, FF, H], dt.bfloat16, kind="ExternalInput")

    # per-feature vectors, host-packed as [128, ntiles] (column j = feats 128j:128j+128)
    # order: bq bk bv bo g1 b1 bo2 g2 b2 pad
    vecs = nc.dram_tensor("vecs", [n_layers, 128, 10 * HT], dt.float32, kind="ExternalInput")
    bi_all = nc.dram_tensor("bi_all", [n_layers, 128, FT], dt.float32, kind="ExternalInput")
    # order: up_b1 up_b2 ln_g ln_b dn_b1 pad
    gvec = nc.dram_tensor("gvec", [128, 6 * HT], dt.float32, kind="ExternalInput")
    dn_b2 = nc.dram_tensor("dn_b2", [128, 1], dt.float32, kind="ExternalInput")

    y = nc.dram_tensor("y", [C, T], dt.float32, kind="ExternalOutput")

    # halo exchange buffers: computed K and V edge tiles (256 tokens per side)
    KSEG = HT * 128 * W1
    VSEG = 2 * VW * 128
    SIDE = KSEG + VSEG
    cc_send = nc.dram_tensor("cc_send", [2 * SIDE], dt.bfloat16)
    cc_recv = nc.dram_tensor("cc_recv", [2, 2 * SIDE], dt.bfloat16)

    with tile.TileContext(nc) as tc, ExitStack() as ctx:
        pp = ctx.enter_context(tc.tile_pool(name="persist", bufs=1))
        wp = ctx.enter_context(tc.tile_pool(name="w768", bufs=6))
        bp = ctx.enter_context(tc.tile_pool(name="bias", bufs=2))
        fp = ctx.enter_context(tc.tile_pool(name="ffmid", bufs=2))
        sp = ctx.enter_context(tc.tile_pool(name="scratch", bufs=1))
        rp = ctx.enter_context(tc.tile_pool(name="rows", bufs=2))
        pbp = ctx.enter_context(tc.tile_pool(name="pbuf", bufs=3))
        psA = ctx.enter_context(tc.tile_pool(name="psA", bufs=2, space="PSUM"))

        # ---------------- persistent tiles ----------------
        h = pp.tile([128, HT * T], dt.float32, tag="h")          # residual stream
        hb = pp.tile([128, HT * T], dt.bfloat16, tag="hb")       # bf16 copy of stream
        qb = pp.tile([128, HT * T], dt.bfloat16, tag="qb")       # Q (feature-major)
        ob = pp.tile([128, HT * T], dt.bfloat16, tag="ob")       # attn out (feature-major)
        # K padded, feature-major, split [left pad | interior | right pad]
        Kpl = pp.tile([128, HT * W1], dt.bfloat16, tag="Kpl")
        Kpm = pp.tile([128, HT * T], dt.bfloat16, tag="Kpm")
        Kpr = pp.tile([128, HT * W1], dt.bfloat16, tag="Kpr")
        # V padded, token-major (65-wide head slots), split by token tiles 0-1|2-9|10-11
        Vpl = pp.tile([128, 2 * VW], dt.bfloat16, tag="Vpl")
        Vpm = pp.tile([128, 8 * VW], dt.bfloat16, tag="Vpm")
        Vpr = pp.tile([128, 2 * VW], dt.bfloat16, tag="Vpr")
        msk = pp.tile([128, 2 * BW], dt.bfloat16, tag="msk")     # banded additive masks
        ones_c = pp.tile([128, 1], dt.bfloat16, tag="ones_c")    # [128,1] ones (stats lhsT)
        ones_r = pp.tile([128, 128], dt.bfloat16, tag="ones_r")  # ones (bcast lhsT slices)
        gv = pp.tile([128, 6 * HT], dt.float32, tag="gv")
        dnb2 = pp.tile([128, 1], dt.float32, tag="dnb2")
        epsc = pp.tile([128, 1], dt.float32, tag="epsc")

        def hs(ft, qt=None):
            if qt is None:
                return slice(ft * T, (ft + 1) * T)
            return slice(ft * T + qt * 512, ft * T + qt * 512 + 512)

        def k_ap(ft, kt):  # lhsT AP [128, 128] for padded key tile kt (0..11)
            if kt < 2:
                return Kpl[:, ft * W1 + kt * 128: ft * W1 + (kt + 1) * 128]
            if kt < 10:
                return Kpm[:, ft * T + (kt - 2) * 128: ft * T + (kt - 1) * 128]
            return Kpr[:, ft * W1 + (kt - 10) * 128: ft * W1 + (kt - 9) * 128]

        def v_ap(i):  # full [128, VW] token tile i of padded V (0..11)
            if i < 2:
                return Vpl[:, i * VW:(i + 1) * VW]
            if i < 10:
                return Vpm[:, (i - 2) * VW:(i - 1) * VW]
            return Vpr[:, (i - 10) * VW:(i - 9) * VW]

        nc.vector.memset(ones_c[:], 1.0)
        nc.vector.memset(ones_r[:], 1.0)
        nc.vector.memset(epsc[:], EPS)
        for i in range(12):
            vre = v_ap(i).rearrange("p (n c) -> p n c", c=VH)
            nc.vector.memset(vre[:, :, HD:HD + 1], 1.0)

        nc.sync.dma_start(out=gv[:], in_=gvec[:])
        nc.sync.dma_start(out=dnb2[:], in_=dn_b2[:])
        nc.sync.dma_start(out=msk[:], in_=mks[:])

        # ---------------- helpers ----------------
        def w_proj(w_dram, rhs_cols, out_fn, n_k=HT, nq=2, qt_post=None,
                   qt_mid=None):
            """psum[mt][qt] = W.T @ rhs ; out_fn(mt, qt, psum) evicts.
            qt-outer so per-half hooks can overlap the other half."""
            wt = []
            for k in range(n_k):
                t = wp.tile([128, H], dt.bfloat16, tag="w768", bufs=8)
                nc.sync.dma_start(out=t[:], in_=w_dram[k * 128:(k + 1) * 128, :])
                wt.append(t)
            for qt in range(nq):
                for mt in range(HT):
                    pt = psA.tile([128, 512], dt.float32, tag="work")
                    for k in range(n_k):
                        nc.tensor.matmul(
                            pt[:], wt[k][:, mt * 128:(mt + 1) * 128],
                            rhs_cols(k, qt),
                            start=(k == 0), stop=(k == n_k - 1))
                    out_fn(mt, qt, pt)
                    if qt_mid is not None:
                        qt_mid(qt, mt)
                if qt_post is not None:
                    qt_post(qt)

        # layernorm for one 512-token half, split into stats (serial scalar
        # chain) and apply (PE broadcasts + row updates) so the chain can hide
        # behind unrelated PE work emitted between the two parts.
        # rsqrt via Act Sqrt + DVE fast reciprocal (single act-table function).
        # h gets g*rsqrt*(h-mean); +b lands on hb via act bias and on h via
        # the Pool engine off the critical path.
        def ln_stats(qt, tS):
            ubf = sp.tile([128, HT * T], dt.bfloat16, tag="scr")
            ub = ubf[:, qt * HT * 512:(qt + 1) * HT * 512]
            for ft in range(HT):
                nc.scalar.activation(ub[:, ft * 512:(ft + 1) * 512],
                                     h[:, hs(ft, qt)], AF.Copy)
            statm = tS[0:1, :]
            for ft in range(HT):
                nc.tensor.matmul(statm, ones_c[:], ub[:, ft * 512:(ft + 1) * 512],
                                 start=(ft == 0), stop=(ft == HT - 1))
            mrow = rp.tile([1, 512], dt.bfloat16, tag="mrow")
            nc.scalar.activation(mrow[:], statm, AF.Copy, scale=1.0 / H)
            m2 = rp.tile([1, 512], dt.float32, tag="rowA")
            nc.scalar.activation(m2[:], statm, AF.Square, scale=1.0 / H)
            for ft in range(HT):
                nc.scalar.square(ub[:, ft * 512:(ft + 1) * 512],
                                 ub[:, ft * 512:(ft + 1) * 512])
            sqs = tS[32:33, :]
            for ft in range(HT):
                nc.tensor.matmul(sqs, ones_c[:], ub[:, ft * 512:(ft + 1) * 512],
                                 start=(ft == 0), stop=(ft == HT - 1))
            var = rp.tile([1, 512], dt.float32, tag="rowB")
            nc.vector.scalar_tensor_tensor(var[:], sqs, 1.0 / H, m2[:],
                                           OP.mult, OP.subtract)
            sd = rp.tile([1, 512], dt.float32, tag="rowA", name="sd")
            nc.scalar.activation(sd[:], var[:], AF.Sqrt, bias=epsc[0:1, :])
            rf = rp.tile([1, 512], dt.float32, tag="rowB", name="rf")
            nc.vector.reciprocal_approx_fast(rf[:], sd[:])
            rrow = rp.tile([1, 512], dt.bfloat16, tag="mrow", name="rrow")
            nc.vector.tensor_copy(rrow[:], rf[:])
            m2row = rp.tile([1, 512], dt.bfloat16, tag="rowC", name="m2row")
            nc.vector.tensor_mul(m2row[:], mrow[:], rrow[:])
            return rrow, m2row

        def ln_apply(qt, tA, tB, rows, g_col, ng_col, b_col):
            rrow, m2row = rows
            nc.tensor.matmul(tB[:], ones_r[0:1, :], rrow[:], start=True, stop=True)
            nc.tensor.matmul(tA[:], ones_r[0:1, :], m2row[:], start=True, stop=True)
            # evict broadcasts to SBUF right away so the psum slots free fast
            rbs = rp.tile([128, 512], dt.bfloat16, tag="lnrb")
            nc.scalar.activation(rbs[:], tB[:], AF.Copy)
            mgs = rp.tile([128, 512], dt.bfloat16, tag="lnmg")
            nc.scalar.activation(mgs[:], tA[:], AF.Copy)
            for ft in range(HT):
                sl = hs(ft, qt)
                tmp = rp.tile([128, 512], dt.float32, tag="lntmp")
                nc.vector.scalar_tensor_tensor(tmp[:], h[:, sl], g_col(ft),
                                               rbs[:], OP.mult, OP.mult)
                nc.vector.scalar_tensor_tensor(h[:, sl], mgs[:], ng_col(ft),
                                               tmp[:], OP.mult, OP.add)
                nc.scalar.activation(hb[:, sl], h[:, sl], AF.Identity,
                                     bias=b_col(ft))
            for ft in range(HT):
                sl = hs(ft, qt)
                nc.vector.tensor_scalar_add(h[:, sl], h[:, sl], b_col(ft))

        # ---------------- input projection ----------------
        # h <- pos_emb + t_emb  (host-combined), then += up-proj
        for ft in range(HT):
            nc.sync.dma_start(out=h[:, hs(ft)], in_=pe[ft * 128:(ft + 1) * 128, :])
        xb = sp.tile([128, T], dt.bfloat16, tag="xb")
        nc.sync.dma_start(out=xb[:], in_=xT[:])

        w1t = wp.tile([128, H], dt.bfloat16, tag="w768", bufs=8)
        nc.sync.dma_start(out=w1t[:], in_=up_w1[:])
        t1 = sp.tile([128, HT * T], dt.bfloat16, tag="scr")
        for mt in range(HT):
            for qt in range(2):
                pt = psA.tile([128, 512], dt.float32, tag="work")
                nc.tensor.matmul(pt[:], w1t[:, mt * 128:(mt + 1) * 128],
                                 xb[:, qt * 512:qt * 512 + 512], start=True, stop=True)
                nc.scalar.activation(t1[:, hs(mt, qt)], pt[:], AF.Tanh,
                                     bias=gv[:, 0 * HT + mt:0 * HT + mt + 1])

        def up2_out(mt, qt, pt):
            sl = hs(mt, qt)
            nc.vector.scalar_tensor_tensor(
                h[:, sl], pt[:], gv[:, 1 * HT + mt:1 * HT + mt + 1], h[:, sl],
                OP.add, OP.add)
        w_proj(up_w2, lambda k, qt: t1[:, hs(k, qt)], up2_out)

        with tc.tile_pool(name="lnps", bufs=2, space="PSUM") as lp0:
            g0 = lambda ft: gv[:, 2 * HT + ft:2 * HT + ft + 1]
            b0 = lambda ft: gv[:, 3 * HT + ft:3 * HT + ft + 1]
            ng0 = lambda ft: gv[:, 5 * HT + ft:5 * HT + ft + 1]
            for qt in range(2):
                tS = lp0.tile([128, 512], dt.float32, tag="lnS")
                rows = ln_stats(qt, tS)
                tA = lp0.tile([128, 512], dt.float32, tag="lnA")
                tB = lp0.tile([128, 512], dt.float32, tag="lnB")
                ln_apply(qt, tA, tB, rows, g0, ng0, b0)

        # deferred LN apply carried across phase boundaries (hides the serial
        # stats->rsqrt scalar chain behind unrelated PE work)
        pend = [None]

        def flush_pend():
            if pend[0] is not None:
                f, pend[0] = pend[0], None
                f()

        # ---------------- encoder layers ----------------
        for l in range(n_layers):
            bv_t = bp.tile([128, 10 * HT], dt.float32, tag="bvec")
            nc.sync.dma_start(out=bv_t[:], in_=vecs[l])
            bi_t = bp.tile([128, FT], dt.float32, tag="bivec")
            nc.sync.dma_start(out=bi_t[:], in_=bi_all[l])

            def vcol(i, ft):
                return bv_t[:, i * HT + ft:i * HT + ft + 1]

            # --- K/V halo exchange: send computed K/V edge tiles (256 tok) ---
            def emit_halo():
                kre = Kpm[:].rearrange("p (f t) -> p f t", t=T)
                for s, k0, v0 in ((0, 0, 0), (1, T - W1, 6 * VW)):
                    nc.sync.dma_start(
                        out=cc_send[s * SIDE:s * SIDE + KSEG]
                        .rearrange("(p f t) -> p f t", p=128, f=HT),
                        in_=kre[:, :, k0:k0 + W1])
                    nc.sync.dma_start(
                        out=cc_send[s * SIDE + KSEG:(s + 1) * SIDE]
                        .rearrange("(p t) -> p t", p=128),
                        in_=Vpm[:, v0:v0 + 2 * VW])
                nc.gpsimd.collective_compute(
                    "AllGather", OP.bypass, ins=[cc_send[:]], outs=[cc_recv[:]],
                    replica_groups=[[0, 1], [2, 3], [4, 5], [6, 7]])
                # left halo <- rank0's right side ; right halo <- rank1's left side
                nc.sync.dma_start(
                    out=Kpl[:].rearrange("p (f t) -> p f t", t=W1),
                    in_=cc_recv[0, SIDE:SIDE + KSEG]
                    .rearrange("(p f t) -> p f t", p=128, f=HT))
                nc.sync.dma_start(
                    out=Vpl[:],
                    in_=cc_recv[0, SIDE + KSEG:2 * SIDE]
                    .rearrange("(p t) -> p t", p=128))
                nc.sync.dma_start(
                    out=Kpr[:].rearrange("p (f t) -> p f t", t=W1),
                    in_=cc_recv[1, 0:KSEG]
                    .rearrange("(p f t) -> p f t", p=128, f=HT))
                nc.sync.dma_start(
                    out=Vpr[:],
                    in_=cc_recv[1, KSEG:SIDE]
                    .rearrange("(p t) -> p t", p=128))

            # --- K projection (interior only; halo arrives via exchange) ---
            def k_out(mt, qt, pt):
                sl = slice(mt * T + qt * 512, mt * T + qt * 512 + 512)
                nc.vector.tensor_scalar_add(Kpm[:, sl], pt[:], vcol(1, mt))

            def k_post(qt):
                if qt == 0:
                    flush_pend()
            w_proj(Wk[l], lambda k, qt: hb[:, hs(k, qt)], k_out, qt_post=k_post)

            # --- V projection (token-major: h stationary); edge tiles first ---
            wvt = []
            for k in range(HT):
                t = wp.tile([128, H], dt.bfloat16, tag="w768", bufs=8)
                nc.sync.dma_start(out=t[:], in_=Wv[l, k * 128:(k + 1) * 128, :])
                wvt.append(t)

            def v_stat(vt, k):  # stationary h slice for interior v token tile vt
                tt = vt - 2
                return hb[:, k * T + tt * 128:k * T + tt * 128 + 128]

            def v_tile(vt):
                for n0, nn in ((0, 512), (512, 256)):
                    pt = psA.tile([128, 512], dt.float32, tag="work")
                    for k in range(HT):
                        nc.tensor.matmul(
                            pt[:, :nn], v_stat(vt, k), wvt[k][:, n0:n0 + nn],
                            start=(k == 0), stop=(k == HT - 1))
                    dst = v_ap(vt).rearrange("p (n c) -> p n c", c=VH)
                    h0, nh_ = n0 // HD, nn // HD
                    src = pt[:, :nn].rearrange("p (n c) -> p n c", c=HD)
                    nc.scalar.activation(dst[:, h0:h0 + nh_, 0:HD], src[:], AF.Copy)

            for vt in (2, 3, 8, 9):
                v_tile(vt)
            emit_halo()
            for vt in (4, 5, 6, 7):
                v_tile(vt)

            # --- Q projection ---
            def q_out(mt, qt, pt):
                nc.vector.tensor_scalar_add(qb[:, hs(mt, qt)], pt[:], vcol(0, mt))
            w_proj(Wq[l], lambda k, qt: hb[:, hs(k, qt)], q_out)

            # --- attention (banded, deferred softmax normalization) ---
            # ob accumulates UNNORMALIZED per-head PV; per-query reciprocal
            # denominators are broadcast per head-pair and multiplied in after.
            # bv's contribution is folded into bo on the host (bo_eff).
            att_cm = tc.tile_pool(name="attps", bufs=2, space="PSUM")
            att_ps = att_cm.__enter__()
            dnp_cm = tc.tile_pool(name="dnp", bufs=2)
            dnp = dnp_cm.__enter__()
            for qt in range(2):
                for ft in range(HT):
                    rbp = psA.tile([128, 512], dt.float32, tag="work")
                    for r in range(2):
                        hd_i, r0 = 2 * ft + r, r * HD
                        pb = pbp.tile([128, BW], dt.bfloat16, tag="pb")
                        for j in range(8):
                            w, off = BWJ[j], BOFF[j]
                            q0 = ft * T + qt * 512 + BQO[j]
                            sc = att_ps.tile([128, 512], dt.float32, tag="sc",
                                             bufs=4)
                            nc.tensor.matmul(
                                sc[:, 0:w], k_ap(ft, 4 * qt + j)[r0:r0 + HD, :],
                                qb[r0:r0 + HD, q0:q0 + w],
                                start=True, stop=True)
                            nc.vector.tensor_add(
                                pb[:, off:off + w], sc[:, 0:w],
                                msk[:, qt * BW + off:qt * BW + off + w])
                        nc.scalar.activation(pb[:], pb[:], AF.Exp, scale=ISQ)
                        pv = att_ps.tile([VH, 512], dt.float32, tag="pvb", bufs=2)
                        for j in (3, 4, 0, 1, 2, 5, 6, 7):
                            nc.tensor.matmul(
                                pv[:, BQO[j]:BQO[j] + BWJ[j]],
                                v_ap(4 * qt + j)[:, hd_i * VH:(hd_i + 1) * VH],
                                pb[:, BOFF[j]:BOFF[j] + BWJ[j]],
                                start=(j == 3), stop=(j == 7),
                                skip_group_check=True)
                        dn_i = dnp.tile([1, 512], dt.float32, tag="dn_i")
                        nc.scalar.activation(dn_i[:], pv[HD:VH, :], AF.Copy)
                        nc.scalar.activation(ob[r0:r0 + HD, hs(ft, qt)],
                                             pv[0:HD, :], AF.Copy)
                        dn_f = dnp.tile([1, 512], dt.float32, tag="dn_f")
                        nc.vector.reciprocal_approx_fast(dn_f[:], dn_i[:])
                        dn_r = dnp.tile([1, 512], dt.bfloat16, tag="dn_r")
                        nc.vector.tensor_copy(dn_r[:], dn_f[:])
                        nc.tensor.matmul(rbp[r0:r0 + HD, :], ones_r[0:1, 0:HD],
                                         dn_r[:], start=True, stop=True)
                    nc.vector.tensor_mul(ob[:, hs(ft, qt)], ob[:, hs(ft, qt)],
                                         rbp[:])
            dnp_cm.__exit__(None, None, None)
            att_cm.__exit__(None, None, None)

            # --- O projection + residual + LN1, FFN + LN2 ---
            # LN stat rows live in spare partitions of ff2_* psum slots; LN
            # applies run from psA between phases, so the serial chains hide
            # behind the other half's projections / FFN chunks.
            ffn_cm = tc.tile_pool(name="ffps", bufs=1, space="PSUM")
            ffn_ps = ffn_cm.__enter__()

            g1c = lambda ft: vcol(4, ft)
            ng1c = lambda ft: vcol(2, ft)
            b1c = lambda ft: vcol(5, ft)
            g2c = lambda ft: vcol(7, ft)
            ng2c = lambda ft: vcol(9, ft)
            b2c = lambda ft: vcol(8, ft)

            def ln_apply_psA(qt, rows, gc, ngc, bc):
                tA = psA.tile([128, 512], dt.float32, tag="work", name="lnA")
                tB = psA.tile([128, 512], dt.float32, tag="work", name="lnB")
                ln_apply(qt, tA, tB, rows, gc, ngc, bc)

            ln1_rows = {}

            def o_out(mt, qt, pt):
                sl = hs(mt, qt)
                nc.vector.scalar_tensor_tensor(h[:, sl], pt[:], vcol(3, mt), h[:, sl],
                                               OP.add, OP.add)

            def o_post(qt):
                tS = ffn_ps.tile([128, 512], dt.float32,
                                 tag=f"ff2_{2 + qt}", name=f"ln1S_{qt}")
                ln1_rows[qt] = ln_stats(qt, tS)
                if qt == 1:
                    pend[0] = lambda: ln_apply_psA(1, ln1_rows[1], g1c, ng1c, b1c)

            def o_mid(qt, mt):
                if qt == 1 and mt == 1:
                    ln_apply_psA(0, ln1_rows[0], g1c, ng1c, b1c)
            w_proj(Wo[l], lambda k, qt: ob[:, hs(k, qt)], o_out,
                   qt_post=o_post, qt_mid=o_mid)

            # --- FFN ---
            ln2_rows = {}
            for qt in range(2):
                fpt = [ffn_ps.tile([128, 512], dt.float32, tag=f"ff2_{m}", name=f"ff2_{m}") for m in range(HT)]
                for ch in range(6):  # 6 chunks of 4 mid tiles (512 cols of FF)
                    wi_ch = []
                    for k in range(HT):
                        t = wp.tile([128, 512], dt.bfloat16, tag="wi", bufs=8)
                        nc.sync.dma_start(
                            out=t[:],
                            in_=Wi[l, k * 128:(k + 1) * 128, ch * 512:(ch + 1) * 512])
                        wi_ch.append(t)
                    fmid = fp.tile([128, 4 * 512], dt.bfloat16, tag="fmid")
                    for mi in range(4):
                        mt = ch * 4 + mi
                        pt = psA.tile([128, 512], dt.float32, tag="work")
                        for k in range(HT):
                            nc.tensor.matmul(
                                pt[:], wi_ch[k][:, mi * 128:(mi + 1) * 128],
                                hb[:, hs(k, qt)],
                                start=(k == 0), stop=(k == HT - 1))
                        nc.scalar.activation(fmid[:, mi * 512:(mi + 1) * 512], pt[:],
                                             AF.Gelu, bias=bi_t[:, mt:mt + 1])
                    wo2_ch = []
                    for mi in range(4):
                        t = wp.tile([128, H], dt.bfloat16, tag="w768", bufs=8)
                        nc.sync.dma_start(
                            out=t[:],
                            in_=Wo2[l, (ch * 4 + mi) * 128:(ch * 4 + mi + 1) * 128, :])
                        wo2_ch.append(t)
                    for m in range(HT):
                        for mi in range(4):
                            kt = ch * 4 + mi
                            nc.tensor.matmul(
                                fpt[m][:], wo2_ch[mi][:, m * 128:(m + 1) * 128],
                                fmid[:, mi * 512:(mi + 1) * 512],
                                start=(kt == 0), stop=(kt == FT - 1))
                    if ch == 1:
                        if qt == 0:
                            flush_pend()  # LN1 qt1 apply
                        else:
                            ln_apply_psA(0, ln2_rows[0], g2c, ng2c, b2c)
                for m in range(HT):
                    sl = hs(m, qt)
                    nc.vector.scalar_tensor_tensor(h[:, sl], fpt[m][:], vcol(6, m),
                                                   h[:, sl], OP.add, OP.add)
                tS = ffn_ps.tile([128, 512], dt.float32,
                                 tag=f"ff2_{2 + qt}", name=f"ln2S_{qt}")
                ln2_rows[qt] = ln_stats(qt, tS)
            lq1, lb = ln2_rows[1], bv_t
            pend[0] = (lambda rows=lq1, bt=lb:
                       ln_apply_psA(1, rows,
                                    lambda ft: bt[:, 7 * HT + ft:7 * HT + ft + 1],
                                    lambda ft: bt[:, 9 * HT + ft:9 * HT + ft + 1],
                                    lambda ft: bt[:, 8 * HT + ft:8 * HT + ft + 1]))
            ffn_cm.__exit__(None, None, None)

        # ---------------- output projection ----------------
        flush_pend()
        t2 = sp.tile([128, HT * T], dt.bfloat16, tag="scr")

        def d1_out(mt, qt, pt):
            nc.scalar.activation(t2[:, hs(mt, qt)], pt[:], AF.Tanh,
                                 bias=gv[:, 4 * HT + mt:4 * HT + mt + 1])
        w_proj(dn_w1, lambda k, qt: hb[:, hs(k, qt)], d1_out)

        w2t = wp.tile([128, HT * C], dt.bfloat16, tag="w768", bufs=8)
        for k in range(HT):
            nc.sync.dma_start(out=w2t[:, k * C:(k + 1) * C],
                              in_=dn_w2[k * 128:(k + 1) * 128, :])
        for qt in range(2):
            pt = psA.tile([128, 512], dt.float32, tag="work")
            for k in range(HT):
                nc.tensor.matmul(pt[:], w2t[:, k * C:(k + 1) * C],
                                 t2[:, hs(k, qt)], start=(k == 0), stop=(k == HT - 1))
            yo = rp.tile([128, 512], dt.float32, tag="lntmp", name="yout")
            nc.scalar.activation(yo[:], pt[:], AF.Identity, bias=dnb2[:])
            nc.sync.dma_start(out=y[:, qt * 512:qt * 512 + 512], in_=yo[:])

    nc.compile()
    return nc


def _host_prep(inputs, n_layers):
    f32 = np.float32
    x = np.asarray(inputs["x"], f32)
    ts = np.asarray(inputs["timesteps"])
    half = C // 2
    freqs = np.exp(-np.log(10000.0) * np.arange(half, dtype=f32) / half)
    a = ts.astype(f32)[:, None] * freqs[None, :]
    emb0 = np.concatenate([np.cos(a), np.sin(a)], -1).astype(f32)
    t1 = emb0 @ np.asarray(inputs["t_w1"], f32) + np.asarray(inputs["t_b1"], f32)
    t1 = t1 / (1.0 + np.exp(-t1))
    emb = (t1 @ np.asarray(inputs["t_w2"], f32) + np.asarray(inputs["t_b2"], f32)).astype(f32)

    def cvt(w):
        return np.ascontiguousarray(np.asarray(w, f32).astype(bf16))

    def packvec(v, nt):
        return np.ascontiguousarray(np.asarray(v, f32).reshape(nt, 128).T)

    com = dict(
        up_w1=cvt(inputs["up_w1"]), up_w2=cvt(inputs["up_w2"]),
        dn_w1=cvt(inputs["down_w1"]), dn_w2=cvt(inputs["down_w2"]),
        Wq=cvt(inputs["Wq"][:n_layers]), Wk=cvt(inputs["Wk"][:n_layers]),
        Wv=cvt(inputs["Wv"][:n_layers]), Wo=cvt(inputs["Wo"][:n_layers]),
        Wi=cvt(inputs["Wi"][:n_layers]), Wo2=cvt(inputs["Wo2"][:n_layers]),
        dn_b2=np.ascontiguousarray(np.asarray(inputs["down_b2"], f32).reshape(1, C).T),
    )
    # bo slot carries bo + bv @ Wo: the attention out bias bv commutes with
    # softmax normalization and folds through the O projection.
    bo_eff = (np.asarray(inputs["bo"], f32)[:n_layers]
              + np.einsum("lh,lhk->lk", np.asarray(inputs["bv"], f32)[:n_layers],
                          np.asarray(inputs["Wo"], f32)[:n_layers]))
    lvec = {k: np.asarray(inputs[k], f32) for k in
            ("bq", "bk", "g1", "b1", "bo2", "g2", "b2")}
    lvec["bo"] = bo_eff
    lvec["ng1"] = -lvec["g1"]
    lvec["ng2"] = -lvec["g2"]
    vecs = np.stack([
        np.concatenate([packvec(lvec[k][l], HT) for k in
                        ("bq", "bk", "ng1", "bo", "g1", "b1", "bo2", "g2", "b2", "ng2")],
                       axis=1)
        for l in range(n_layers)])
    com["vecs"] = np.ascontiguousarray(vecs.astype(f32))
    com["bi_all"] = np.ascontiguousarray(
        np.stack([packvec(np.asarray(inputs["bi"], f32)[l], FT)
                  for l in range(n_layers)]).astype(f32))
    com["gvec"] = np.ascontiguousarray(np.concatenate([
        packvec(inputs["up_b1"], HT), packvec(inputs["up_b2"], HT),
        packvec(inputs["ln_g"], HT), packvec(inputs["ln_b"], HT),
        packvec(inputs["down_b1"], HT),
        packvec(-np.asarray(inputs["ln_g"], f32), HT)],
        axis=1).astype(f32))

    pos = np.asarray(inputs["pos_emb"], f32)
    mk = {}
    for sh in range(2):
        base = sh * T
        m = np.empty((128, 2 * BW), f32)
        for qt in range(2):
            for j in range(8):
                gk = base - W1 + (4 * qt + j) * 128 + np.arange(128)[:, None]
                gq = base + qt * 512 + BQO[j] + np.arange(BWJ[j])[None, :]
                valid = (np.abs(gk - gq) <= W1) & (gk >= 0) & (gk < S)
                m[:, qt * BW + BOFF[j]:qt * BW + BOFF[j] + BWJ[j]] = \
                    np.where(valid, 0.0, NEG)
        mk[sh] = np.ascontiguousarray(m.astype(bf16))

    in_maps = []
    for c in range(8):
        b, sh = c // 2, c % 2
        sl = slice(sh * T, (sh + 1) * T)
        im = dict(com)
        im["xT"] = np.ascontiguousarray(x[b, sl].T.astype(bf16))
        im["pe"] = np.ascontiguousarray((pos[sl] + emb[b][None, :]).T.astype(f32))
        im["mks"] = mk[sh]
        in_maps.append(im)
    return in_maps


def kernel(**inputs):
    from concourse.bass_utils import run_bass_kernel_spmd

    n_layers = L
    if n_layers not in _CACHED:
        _CACHED[n_layers] = _build(n_layers)
    nc = _CACHED[n_layers]
    in_maps = _host_prep(inputs, n_layers)
    trace = os.environ.get("KERNEL_TRACE", "0") == "1"
    tmpdir = os.environ.get("KERNEL_TMPDIR") or None
    res = run_bass_kernel_spmd(nc, in_maps, list(range(8)), trace=trace,
                               tmpdir=tmpdir)
    global LAST_EXEC_NS
    if getattr(res, "exec_time_ns", None):
        LAST_EXEC_NS = res.exec_time_ns
    out = np.empty((B, S, C), np.float32)
    for c in range(8):
        b, sh = c // 2, c % 2
        out[b, sh * T:(sh + 1) * T, :] = res.results[c]["y"].T
    return out

